# revision 19
# baseline (speedup 1.0000x reference)
"""NextVLAD Trainium2 kernel — 8-way data-parallel over batch (1 sample/core).

v2 dataflow per core (M=512 tokens, N=1024, E*N=2048, G=8, K=128, D=256):
  host packs every tensor in final SBUF partition layout -> few dense DMAs.
  xsq = (x*0.25)*x fp8 (DVE) ; ss = ones-matmul fp8 DoubleRow
  inv = recip_approx(sqrt(ss))            (= 0.25/||x||)
  y chains: 16 e-tiles fp8 DR; DVE writes ybp fp8 (=32*x^W), Pool writes
  y16 bf16; y16 dumped to DRAM, 4 XBAR dma transposes -> yTraw [m,e];
  DVE scatter-add bias -> yT [m, g*BW blocks] bf16 (ones col = 32).
  sg = sigmoid((WgW x)*inv/32 + bg') ; sgc = PE transpose to [m,G]
  phase2: logits fp8 DR (w2 fully prefetched), exp -> ex bf16
  se = ones-matmul ; ise = recip_approx_fast ; wf = (ex*sgc)*ise bf16
  (split DVE/Pool) ; einsum bf16: vd[k, 258] accumulates 8g x 4m
  vlad = vd - S*cent ; out = vlad * rsqrt(sumsq)  (global norm = /sqrt(128))
Activation table order: Sqrt, Sigmoid, Exp, Sqrt with dummy preloads.
"""
import os
import numpy as np

N = 1024          # feature size
EN = 2048         # expanded features
G = 8             # groups
KC = 128          # clusters
D = 256           # per-group cluster dim
BW = D + 2        # group block width in yT (data + ones + pad)
M = 512           # tokens per sample (8*8*8)
ET = EN // 128    # 16 e-tiles
MT = 4            # m-tiles of 128
W1W = EN + G + 8  # padded w1 block (step%16==0 for DoubleRow)

_cache = {}


def _build_nc():
    import concourse.bacc as bacc
    import concourse.tile as tile
    from concourse import mybir

    f32 = mybir.dt.float32
    f32r = mybir.dt.float32r
    bf16 = mybir.dt.bfloat16
    fp8 = mybir.dt.float8e4
    Alu = mybir.AluOpType
    Act = mybir.ActivationFunctionType
    DR = mybir.MatmulPerfMode.DoubleRow

    nc = bacc.Bacc("TRN2", target_bir_lowering=False)
    xt_d = nc.dram_tensor("xt", [128, 4 * 1024], fp8, kind="ExternalInput")
    w1_d = nc.dram_tensor("w1", [128, 4 * 2 * W1W], fp8, kind="ExternalInput")
    w2_d = nc.dram_tensor("w2", [128, 8 * 2 * 1024], fp8, kind="ExternalInput")
    cf_d = nc.dram_tensor("cf", [128, D + 9], f32, kind="ExternalInput")   # -cent|eye8|bg
    crr_d = nc.dram_tensor("crr", [128, 256], f32r, kind="ExternalInput")  # ones|eye128
    binp_d = nc.dram_tensor("binp", [1, EN], f32, kind="ExternalInput")    # 32*b_inp
    out_d = nc.dram_tensor("out", [KC, D], f32, kind="ExternalOutput")

    with tile.TileContext(nc) as tc:
        with tc.tile_pool(name="const", bufs=1) as constp, \
             tc.tile_pool(name="work", bufs=1) as work, \
             tc.tile_pool(name="dram", bufs=1, space="DRAM") as dramp, \
             tc.tile_pool(name="ps", bufs=1, space="PSUM") as ps:
            # ---------------- input DMAs (dense, pre-packed) ----------------
            # separate SBUF tile per chunk: tile-granular deps unlock compute
            # as soon as each chunk's DMA lands (one tile would wait for all)
            xc = [constp.tile([128, 2048], fp8, name=f"xc{i}") for i in range(2)]
            w1c = [constp.tile([128, 2 * W1W], fp8, name=f"w1c{c}") for c in range(4)]
            w2c = [constp.tile([128, 8192], fp8, name=f"w2c{i}") for i in range(2)]
            crb = constp.tile([128, 128], bf16, name="crb")
            cf = constp.tile([128, D + 9], f32, name="cf")
            crr = constp.tile([128, 256], f32r, name="crr")
            binp_b = constp.tile([128, EN], f32, name="binp")
            # x + w1 first (sync/scalar queues); w2/binp deferred to gpsimd
            # so they don't steal HBM bandwidth from the critical path
            nc.sync.dma_start(out=xc[0][:], in_=xt_d[:, 0:2048])
            nc.scalar.dma_start(out=xc[1][:], in_=xt_d[:, 2048:4096])
            for c in range(4):
                eng = nc.sync if c % 2 == 0 else nc.scalar
                eng.dma_start(out=w1c[c][:],
                              in_=w1_d[:, c * 2 * W1W:(c + 1) * 2 * W1W])
            nc.vector.memset(crb[:], 1.0)
            nc.gpsimd.dma_start(out=crr[:], in_=crr_d[:])
            nc.gpsimd.dma_start(out=cf[:], in_=cf_d[:])
            # gate w2/binp transfers behind x arrival so x+w1 get the HBM
            # bandwidth exclusively during the critical startup window
            w2gate = dramp.tile([1, 64], fp8, name="w2gate")
            nc.gpsimd.dma_start(out=w2gate[:], in_=xc[1][0:1, 1984:2048])
            nc.gpsimd.dma_start(out=w2c[0][:], in_=w2_d[:, 0:8192])
            nc.gpsimd.dma_start(out=w2c[1][:], in_=w2_d[:, 8192:16384])
            nc.gpsimd.dma_start(out=binp_b[:], in_=binp_d[:].to_broadcast([128, EN]))
            centn_t = cf[:, 0:D]
            identf8_t = cf[0:G, D:D + G]
            bg_t = cf[0:G, D + G:D + G + 1]
            ones_r = crr[:, 0:128]
            identr_t = crr[:, 128:256]

            xv = [t.rearrange("p (c s m) -> p c s m", c=2, m=M) for t in xc]
            w1v = [t.rearrange("p (s e) -> p s e", e=W1W) for t in w1c]
            w2v = [t.rearrange("p (q s j) -> p q s j", q=4, j=1024) for t in w2c]

            def xch(c):
                return xv[c // 2][:, c % 2]


            dum = work.tile([1, 1], f32, name="dum")
            nc.vector.memset(dum[:], 1.0)
            # ---------------- phase 1: ss -> inv ----------------
            xsq = work.tile([128, 4096], bf16, name="xsq")
            xsqv = xsq.rearrange("p (c s m) -> p c s m", c=4, m=M)
            for c in range(4):
                xf = xc[c // 2][:, (c % 2) * 1024:(c % 2 + 1) * 1024]
                nc.vector.tensor_mul(xsq[:, c * 1024:(c + 1) * 1024], xf, xf)
            ss_ps = ps.tile([128, M], f32, name="ss_ps", tag="ps", bufs=8)
            for c in range(4):
                for s in range(2):
                    nc.tensor.matmul(ss_ps[:], crb[:], xsqv[:, c, s],
                                     start=(c == 0 and s == 0),
                                     stop=(c == 3 and s == 1))
            nrm_t = work.tile([128, M], f32r, name="nrm")
            nc.scalar.activation(nrm_t[:], ss_ps[:], Act.Sqrt, scale=0.25)  # 4*||x||
            inv_t = work.tile([128, M], f32, name="inv")         # = 0.25/||x||
            nc.vector.reciprocal_approx_fast(out=inv_t[:], in_=nrm_t[:].bitcast(f32))
            # per-token inv in [m,*] layout: PE-transpose the (row-identical)
            # nrm tile; column 0 of each transposed block is nrm per-token
            nrt_ps = ps.tile([128, 512], f32, name="nrt_ps", tag="ps", bufs=8)
            for t in range(MT):
                nc.tensor.transpose(nrt_ps[:, t * 128:(t + 1) * 128].bitcast(f32r),
                                    nrm_t[:, t * 128:(t + 1) * 128],
                                    identr_t)
            nrmP = work.tile([128, MT], f32, name="nrmP")
            nc.vector.tensor_copy(nrmP[:],
                                  nrt_ps.rearrange("p (t j) -> p t j", j=128)[:, :, 0])
            invP = work.tile([128, MT], f32, name="invP")
            nc.vector.reciprocal_approx_fast(out=invP[:], in_=nrmP[:])
            invP2 = work.tile([128, MT], f32, name="invP2")   # = 0.5/||x||
            nc.vector.tensor_scalar_mul(invP2[:], invP[:], 2.0)
            invPe = work.tile([128, MT], f32, name="invPe")   # = 1/(512||x||)
            nc.vector.tensor_scalar_mul(invPe[:], invP[:], 1.0 / 128.0)

            # ---------------- phase 1: y chains ----------------
            ybp = [work.tile([128, 2 * M], fp8, name=f"ybp{c}") for c in range(ET // 2)]
            grp = [list(range(0, 6)), list(range(6, 12)), list(range(12, 16))]
            y16 = [work.tile([128, len(g) * M], bf16, name=f"y16{i}")
                   for i, g in enumerate(grp)]
            y16d = dramp.tile([EN, M], bf16, name="y16d")

            def post_e(e, yp):
                # raw y_ps (=64*W'x) out both ways; inv is applied later
                # (exp AP-scale for phase 2, invP2 post-transpose for yT)
                sw, idx = e // 6, e % 6
                nc.scalar.activation(ybp[e // 2][:, (e % 2) * M:(e % 2 + 1) * M],
                                     yp[:], Act.Copy)
                nc.vector.tensor_copy(y16[sw][:, idx * M:(idx + 1) * M], yp[:])

            def dump(sw):
                g = grp[sw]
                lo = g[0] * 128
                nc.sync.dma_start(
                    out=y16d[lo:lo + len(g) * 128, :].rearrange("(t p) m -> p t m", p=128),
                    in_=y16[sw].rearrange("p (t m) -> p t m", m=M))

            # sweep A (chunk-major, paced by w1 chunk arrival): sg + e0..e5
            es = grp[0]
            sg_ps = ps.tile([G, M], f32, name="sg_ps", tag="ps", bufs=8)
            ysw_ps = [ps.tile([128, M], f32, name=f"y_ps{e}", tag="ps", bufs=8)
                      for e in es]
            for c in range(4):
                nc.tensor.matmul(sg_ps[:], w1v[c][:, :, EN:EN + G], xch(c),
                                 start=(c == 0), stop=(c == 3), perf_mode=DR)
                for k, e in enumerate(es):
                    nc.tensor.matmul(ysw_ps[k][:], w1v[c][:, :, e * 128:(e + 1) * 128],
                                     xch(c), start=(c == 0), stop=(c == 3),
                                     perf_mode=DR)
            # sigmoid gate chain immediately (clears PSUM ring before phase 2)
            sgs_t = work.tile([G, M], f32, name="sgs")
            sgc_t = [work.tile([128, G], f32, name=f"sgc{m}") for m in range(MT)]
            with tc.high_priority():
                nc.vector.scalar_tensor_tensor(out=sgs_t[:], in0=sg_ps[:],
                                               scalar=1.0 / 16.0, in1=inv_t[0:G, :],
                                               op0=Alu.mult, op1=Alu.mult)
                nc.scalar.activation(sgs_t[:], sgs_t[:], Act.Sigmoid, bias=bg_t[:])
                for m in range(MT):
                    sgc_ps = ps.tile([128, G], f32, name="sgc_ps", tag="ps", bufs=8)
                    nc.tensor.matmul(sgc_ps[:], sgs_t[:, m * 128:(m + 1) * 128],
                                     identf8_t, start=True, stop=True)
                    nc.vector.tensor_copy(sgc_t[m][:], sgc_ps[:])
            for k, e in enumerate(es):
                post_e(e, ysw_ps[k])
            dump(0)
            # remaining chains chain-major: each finishes fast so the scalar
            # fp8 copy pipelines at chain cadence instead of bursting at the end
            for e in range(6, ET):
                yp = ps.tile([128, M], f32, name=f"y_ps{e}", tag="ps", bufs=8)
                for c in range(4):
                    nc.tensor.matmul(yp[:], w1v[c][:, :, e * 128:(e + 1) * 128],
                                     xch(c), start=(c == 0), stop=(c == 3),
                                     perf_mode=DR)
                post_e(e, yp)
                if e == 11:
                    dump(1)
            dump(2)

            # ---------------- transposes: 4 XBAR DMAs + bias scatter ----------------
            yTraw = [work.tile([128, EN], bf16, name=f"yTraw{m}") for m in range(MT)]
            yT_t = [work.tile([128, G * BW], bf16, name=f"yT{m}") for m in range(MT)]
            for m in range(MT):
                nc.sync.dma_start_transpose(
                    out=yTraw[m][:], in_=y16d[:, m * 128:(m + 1) * 128])
            for m in range(MT):
                # ones col (=32, matches global 32 scale) and zero pad col
                nc.gpsimd.memset(yT_t[m].rearrange("p (g c) -> p g c", c=BW)[:, :, D:D + 1], 32.0)
                nc.gpsimd.memset(yT_t[m].rearrange("p (g c) -> p g c", c=BW)[:, :, D + 1:D + 2], 0.0)
                nc.vector.scalar_tensor_tensor(
                    out=yT_t[m].rearrange("p (g c) -> p g c", c=BW)[:, :, 0:D]
                              .rearrange("p g (h j) -> p g h j", j=128),
                    in0=yTraw[m].rearrange("p (g h j) -> p g h j", g=G, j=128),
                    scalar=invP2[:, m:m + 1],
                    in1=binp_b.rearrange("p (g h j) -> p g h j", g=G, j=128),
                    op0=Alu.mult, op1=Alu.add)

            # ---------------- phase 2: gk logits + exp ----------------
            ex_t = [[work.tile([128, 512], f32r, name=f"ex{m}_{h}") for h in range(2)]
                    for m in range(MT)]
            lg_ps = [[ps.tile([128, 512], f32, name=f"lg{m}_{h}", tag="ps", bufs=8)
                      for h in range(2)] for m in range(MT)]
            ybv = [t.rearrange("p (s m) -> p s m", m=M) for t in ybp]
            LAG = 2
            for e2 in range(8 + LAG):
                if e2 < 8:
                    for m in range(MT):
                        nc.tensor.matmul(lg_ps[m][0][:],
                                         ybv[e2][:, :, m * 128:(m + 1) * 128],
                                         w2v[e2 // 4][:, e2 % 4, :, 0:512],
                                         start=(e2 == 0), stop=(e2 == 7), perf_mode=DR)
                    if e2 == 7:
                        for m in range(MT):
                            nc.scalar.activation(ex_t[m][0][:], lg_ps[m][0][:],
                                                 Act.Exp, scale=invPe[:, m:m + 1])
                eh = e2 - LAG
                if eh >= 0:
                    for m in range(MT):
                        nc.tensor.matmul(lg_ps[m][1][:],
                                         ybv[eh][:, :, m * 128:(m + 1) * 128],
                                         w2v[eh // 4][:, eh % 4, :, 512:1024],
                                         start=(eh == 0), stop=(eh == 7), perf_mode=DR)
                    if eh == 7:
                        for m in range(MT):
                            nc.scalar.activation(ex_t[m][1][:], lg_ps[m][1][:],
                                                 Act.Exp, scale=invPe[:, m:m + 1])
            # final-Sqrt table preload: depends on last ex tile so the
            # scheduler cannot hoist it before the exps
            nc.scalar.activation(dum[:], ex_t[3][1][0:1, 0:1].bitcast(f32), Act.Sqrt)

            # ---------------- phase 3: softmax denom, weights, einsum ----------------
            ise_t = [work.tile([128, 512], f32, name=f"ise{h}") for h in range(2)]
            wf_t = [[work.tile([128, KC], bf16, name=f"wf{m}_{g}") for g in range(G)]
                    for m in range(MT)]
            vd_ps = ps.tile([128, 512], f32, name="vd_ps", tag="ps", bufs=8)[:, 0:BW]

            k = 0
            for h in range(2):
                se_ps = ps.tile([128, 512], f32, name=f"se{h}", tag="ps", bufs=8)
                for m in range(MT):
                    nc.tensor.matmul(se_ps[:], ones_r, ex_t[m][h][:],
                                     start=(m == 0), stop=(m == MT - 1))
                nc.vector.reciprocal_approx_fast(out=ise_t[h][:], in_=se_ps[:])
                for g in range(h * 4, h * 4 + 4):
                    lc = g * KC - h * 512
                    for m in range(MT):
                        nc.vector.scalar_tensor_tensor(
                            out=wf_t[m][g][:],
                            in0=ex_t[m][h][:, lc:lc + KC].bitcast(f32),
                            scalar=sgc_t[m][:, g:g + 1], in1=ise_t[h][:, lc:lc + KC],
                            op0=Alu.mult, op1=Alu.mult)
                    for m in range(MT):
                        nc.tensor.matmul(vd_ps[:], wf_t[m][g][:],
                                         yT_t[m][:, g * BW:(g + 1) * BW],
                                         start=(k == 0), stop=(k == G * MT - 1))
                        k += 1

            # ---------------- final: centroid fixup + l2 norm ----------------
            vlad_t = work.tile([128, D], f32, name="vlad")
            nc.vector.scalar_tensor_tensor(
                out=vlad_t[:], in0=centn_t[:], scalar=vd_ps[:, D:D + 1],
                in1=vd_ps[:, 0:D], op0=Alu.mult, op1=Alu.add)
            sq_t = work.tile([128, D], f32, name="sq")
            ss2_t = work.tile([128, 1], f32, name="ss2")
            nc.vector.scalar_tensor_tensor(
                out=sq_t[:], in0=vlad_t[:], scalar=1.0, in1=vlad_t[:],
                op0=Alu.bypass, op1=Alu.mult, accum_out=ss2_t[:])
            nr2_t = work.tile([128, 1], f32, name="nr2")
            nc.scalar.activation(nr2_t[:], ss2_t[:], Act.Sqrt, scale=128.0)
            r1_t = work.tile([128, 1], f32, name="r1")
            nc.vector.reciprocal_approx_fast(out=r1_t[:], in_=nr2_t[:])
            out_t = work.tile([128, D], f32, name="out")
            nc.vector.tensor_scalar_mul(out_t[:], vlad_t[:], r1_t[:])
            nc.sync.dma_start(out=out_d[:], in_=out_t[:])

    nc.compile()
    return nc


def _get_nc():
    if "nc" not in _cache:
        _cache["nc"] = _build_nc()
    return _cache["nc"]


def kernel(x, W_inp, b_inp, W_g, b_g, W_gk, b_gk, centroids):
    from concourse.bass_utils import run_bass_kernel_spmd
    import ml_dtypes as mld

    nc = _get_nc()

    x = np.asarray(x, dtype=np.float32)
    X = x.reshape(8, 8, N, 64).transpose(0, 2, 1, 3).reshape(8, N, M)
    WgT = ((np.asarray(W_g, np.float64) @ np.asarray(W_inp, np.float64)).T
           ).astype(np.float32)
    W1 = np.zeros((N, W1W), np.float32)
    W1[:, 0:EN] = np.asarray(W_inp, np.float32).T
    W1[:, EN:EN + G] = WgT
    W1 = np.ascontiguousarray(
        (W1 * 8.0).reshape(4, 2, 128, W1W).transpose(2, 0, 1, 3)
        .reshape(128, 4 * 2 * W1W).astype(mld.float8_e4m3))
    W2 = np.ascontiguousarray(
        (np.asarray(W_gk, np.float32).T * 8.0)
        .reshape(8, 2, 128, 1024).transpose(2, 0, 1, 3)
        .reshape(128, 8 * 2048).astype(mld.float8_e4m3))
    bg = (np.asarray(b_g, np.float64)
          + np.asarray(W_g, np.float64) @ np.asarray(b_inp, np.float64)
          ).astype(np.float32)
    binp = np.ascontiguousarray(
        np.asarray(b_inp, np.float32).reshape(1, EN) * 32.0)
    cf = np.zeros((128, D + 9), np.float32)
    cf[:, 0:D] = -np.asarray(centroids, np.float32)
    cf[0:G, D:D + G] = np.eye(G, dtype=np.float32)
    cf[0:G, D + G] = bg
    crr = np.concatenate([np.ones((128, 128), np.float32),
                          np.eye(128, dtype=np.float32)], axis=1)

    in_maps = []
    for b in range(8):
        xb = np.ascontiguousarray(
            (X[b] * 8.0).reshape(4, 2, 128, M).transpose(2, 0, 1, 3)
            .reshape(128, 4096).astype(mld.float8_e4m3))
        in_maps.append({
            "xt": xb, "w1": W1, "w2": W2, "cf": cf, "crr": crr, "binp": binp,
        })

    trace = os.environ.get("KERNEL_TRACE") == "1"
    r = run_bass_kernel_spmd(nc, in_maps, core_ids=list(range(8)), trace=trace)
    _cache["last_results"] = r
    return np.stack([r.results[b]["out"].reshape(KC * D) for b in range(8)]).astype(np.float32)
